# revision 6
# baseline (speedup 1.0000x reference)
"""Trainium2 Bass kernel for nn_LocalNet (binary-tree reduction network).

Computation: x [2048, 65536] f32; 16 levels of per-pair Linear(2,1) + ReLU
(no ReLU on the last level) -> out [2048, 1].

Strategy (pure data parallel, 8 cores, 256 rows each):
- Host: within each 512-feature partition block, permute columns by 9-bit
  bit-reversal.  This makes every tree level "planar": pair partners sit at
  (i, i + n/2), so all DVE accesses are unit-stride and fp16 tensor_tensor
  runs in 2x mode.
- Device, per core: stream groups of R rows as [128, R, 512] f32 tiles
  (partition p holds that row's features [512p, 512p+512) bitrev-permuted),
  cast to fp16 on ScalarE, then levels 0..8 on VectorE:
      z = s * wcat        (wcat = [W0 | W1] planar, broadcast over rows)
      v = z[:half] + z[half:]
      s' = relu(v)
  Level-8 outputs accumulate into a [128, 256] staging tile (node q of each
  row's 128-node level-9 input lives on partition q).  Two 128x128 DMA-xbar
  transposes flip rows onto partitions; levels 9..15 then run along the free
  axis with host-replicated weights.  Final [256,1] f32 DMAed out per core.
"""

import sys

for _p in ("/opt/trn_rl_repo",):
    if _p not in sys.path:
        sys.path.insert(0, _p)

import numpy as np

TREE_DEPTH = 16
BATCH = 2048
FEATS = 65536
NCORES = 8
ROWS = BATCH // NCORES      # 256 rows per core
P = 128                     # SBUF partitions
SUB = FEATS // P            # 512 features per partition subtree
R = 16                      # rows per streamed group
G = ROWS // R               # 16 groups
IN_LEVELS = 9               # levels 0..8 run inside partitions
F16 = "float16"


def _bitrev_array(bits):
    n = 1 << bits
    r = np.zeros(n, dtype=np.int64)
    for i in range(n):
        v = 0
        for b in range(bits):
            if i & (1 << b):
                v |= 1 << (bits - 1 - b)
        r[i] = v
    return r


def _host_pack(x, weights):
    """Build per-core input arrays + shared weight arrays."""
    brev = _bitrev_array(9)
    xs = np.ascontiguousarray(x, dtype=np.float32).reshape(BATCH, P, SUB)[:, :, brev]
    per_core_x = []
    for c in range(NCORES):
        xc = np.ascontiguousarray(xs[c * ROWS:(c + 1) * ROWS].transpose(1, 0, 2))
        per_core_x.append(xc)  # [128, 256, 512] f32

    blocks = []
    for l in range(IN_LEVELS):
        n = SUB >> l
        half = n // 2
        idx = _bitrev_array(8 - l) if half > 1 else np.zeros(1, dtype=np.int64)
        W = np.asarray(weights[l])                      # [2^(15-l), 2]
        q = np.arange(P)[:, None]
        g = q * half + idx[None, :]
        W0 = W[g, 0].astype(np.float16)
        W1 = W[g, 1].astype(np.float16)
        blocks.append(np.concatenate([W0, W1], axis=1))  # [128, n] fp16
    for l in range(IN_LEVELS, TREE_DEPTH):
        m = 1 << (15 - l)
        W = np.asarray(weights[l]).astype(np.float16)    # [m, 2]
        blocks.append(np.broadcast_to(W[None, :, 0], (P, m)))
        blocks.append(np.broadcast_to(W[None, :, 1], (P, m)))
    wall = np.ascontiguousarray(np.concatenate(blocks, axis=1))  # [128, WTOT]
    wparams = {"wall": wall}
    return per_core_x, wparams


_CACHED = {}


def _build():
    """Build the (SPMD-identical) Bass module once."""
    if "nc" in _CACHED:
        return _CACHED["nc"]
    import concourse.bacc as bacc
    import concourse.mybir as mybir
    import concourse.tile as tile

    dt = mybir.dt
    nc = bacc.Bacc(
        "TRN2", target_bir_lowering=False, debug=False, num_devices=NCORES)

    xin = nc.declare_dram_parameter("x", [P, ROWS, SUB], dt.float32, isOutput=False)
    WTOT = sum(SUB >> l for l in range(IN_LEVELS)) + 2 * sum(
        1 << (15 - l) for l in range(IN_LEVELS, TREE_DEPTH))
    wallp = nc.declare_dram_parameter("wall", [P, WTOT], dt.float16, isOutput=False)
    out = nc.declare_dram_parameter("out", [ROWS, 1], dt.float32, isOutput=True)

    with tile.TileContext(nc) as tc:
        with (
            tc.tile_pool(name="xio", bufs=2) as xpool,
            tc.tile_pool(name="sio", bufs=2) as spool,
            tc.tile_pool(name="early", bufs=2) as early,
            tc.tile_pool(name="deep", bufs=1) as deep,
            tc.tile_pool(name="weights", bufs=1) as wp,
            tc.tile_pool(name="stage", bufs=1) as stp,
        ):
            def wkpool(l):
                return early if l == 0 else deep
            # Load all weights in ONE DMA; slice views per level.
            wtile = wp.tile([P, WTOT], dt.float16, tag="wall")
            nc.sync.dma_start(out=wtile[:], in_=wallp[:])
            wtiles = []
            off = 0
            for l in range(IN_LEVELS):
                n = SUB >> l
                wtiles.append(wtile[:, off:off + n])
                off += n
            twt = {}
            for l in range(IN_LEVELS, TREE_DEPTH):
                m = 1 << (15 - l)
                a = wtile[:, off:off + m]; off += m
                b = wtile[:, off:off + m]; off += m
                twt[l] = (a, b)

            staging = stp.tile([P, ROWS, 1], dt.float16, tag="staging")

            for g in range(G):
                xg = xpool.tile([P, R, SUB], dt.float32, tag="xg")
                nc.sync.dma_start(out=xg[:], in_=xin[:, g * R:(g + 1) * R, :])
                s = spool.tile([P, R, SUB], dt.float16, tag="s0")
                nc.scalar.copy(out=s[:], in_=xg[:])
                for l in range(IN_LEVELS):
                    n = SUB >> l
                    half = n // 2
                    pl = wkpool(l)
                    z = pl.tile([P, R, n], dt.float16, tag=f"z{l}")
                    wb = (wtiles[l]
                          .rearrange("p (o n) -> p o n", o=1)
                          .broadcast_to([P, R, n]))
                    nc.vector.tensor_mul(z[:], s[:], wb)
                    if l < IN_LEVELS - 1:
                        v = pl.tile([P, R, half], dt.float16, tag=f"v{l}")
                        nc.vector.tensor_add(v[:], z[:, :, :half], z[:, :, half:])
                        sn = pl.tile([P, R, half], dt.float16, tag=f"s{l + 1}")
                        nc.vector.tensor_scalar_max(sn[:], v[:], 0.0)
                        s = sn
                    else:
                        # level 8: half == 1 -> into staging (with relu)
                        v = deep.tile([P, R, 1], dt.float16, tag="v8")
                        nc.vector.tensor_add(v[:], z[:, :, 0:1], z[:, :, 1:2])
                        nc.vector.tensor_scalar_max(
                            staging[:, g * R:(g + 1) * R, :], v[:], 0.0)

            # ---- tail: levels 9..15 ----
            st2 = staging[:].rearrange("p r one -> p (r one)")  # [128, 256] fp16
            tt = []
            for hblk in range(2):
                tb = stp.tile([P, P], dt.float16, tag=f"tr{hblk}")
                nc.sync.dma_start_transpose(
                    out=tb[:], in_=st2[:, hblk * P:(hblk + 1) * P])
                tt.append(tb)

            ofin = stp.tile([P, 2], dt.float32, tag="ofin")
            for hblk in range(2):
                t = tt[hblk]  # [128 rows, 128 nodes] fp16
                cur = t
                width = P
                for l in range(IN_LEVELS, TREE_DEPTH):
                    m = width // 2
                    a, b = twt[l]
                    z0 = stp.tile([P, m], dt.float16, tag=f"tz0_{l}_{hblk}")
                    z1 = stp.tile([P, m], dt.float16, tag=f"tz1_{l}_{hblk}")
                    nc.vector.tensor_mul(z0[:], cur[:, 0:width:2], a)
                    nc.vector.tensor_mul(z1[:], cur[:, 1:width:2], b)
                    if l < TREE_DEPTH - 1:
                        vv = stp.tile([P, m], dt.float16, tag=f"tv_{l}_{hblk}")
                        nc.vector.tensor_add(vv[:], z0[:], z1[:])
                        nxt = stp.tile([P, m], dt.float16, tag=f"ts_{l}_{hblk}")
                        nc.vector.tensor_scalar_max(nxt[:], vv[:], 0.0)
                        cur = nxt
                        width = m
                    else:
                        nc.vector.tensor_add(
                            ofin[:, hblk:hblk + 1], z0[:], z1[:])

            # out[r] for r in 0..127 from ofin[:,0]; 128..255 from ofin[:,1]
            for hblk in range(2):
                nc.sync.dma_start(
                    out=out[hblk * P:(hblk + 1) * P, :],
                    in_=ofin[:, hblk:hblk + 1])

    nc.compile()
    _CACHED["nc"] = nc
    return nc


def kernel(x, weights):
    from concourse.bass_utils import run_bass_kernel_spmd

    per_core_x, wparams = _host_pack(x, weights)
    nc = _build()
    in_maps = []
    for c in range(NCORES):
        m = {"x": per_core_x[c]}
        m.update(wparams)
        in_maps.append(m)
    res = run_bass_kernel_spmd(nc, in_maps, list(range(NCORES)))
    outs = [res.results[c]["out"] for c in range(NCORES)]
    return np.concatenate(outs, axis=0).astype(np.float32)
